# revision 14
# baseline (speedup 1.0000x reference)
"""NetVLAD pooling kernel for Trainium2 (Bass/Tile), 8-core data-parallel.

Reference computation (per batch b):
    scores = conv_w @ x[b]                  # [K, N]
    assign = softmax(scores, axis=K)
    vlad   = x[b] @ assign.T - centers * assign.sum(n)   # [D, K]
    vlad  /= max(||vlad||_2 over D, eps)    # intra-norm per cluster column
    desc   = vlad.reshape(D*K) / max(||.||_2, eps)

Shapes: x [32, 512, 1024] f32, conv_w [64, 512], centers [512, 64],
output desc [32, 32768] f32.  Sharding: data-parallel over batch,
4 batches per core; params replicated.

v2 design (bf16 PE path; v1 was f32r with PE transposes of x and E):

  * x is cast f32->bf16 *during* the DMA (SWDGE on gpsimd), in 8
    half-batch chunks so compute pipelines behind the load.
  * scores are computed TRANSPOSED: sT[n,k] = sum_d x[d,n] wT[d,k] with
    the x chunk [d=128, n=128] as the PE stationary operand.  The same
    stationary chunk then streams the identity to produce xT[n,d] - the
    transpose of x falls out of the weight loads the scores matmul
    already pays for, and the per-batch E^T transposes of v1 vanish
    because softmax-over-k is now a free-dim reduce in [n,k] layout.
  * softmax reciprocal rec[n] = 1/sum_k exp(sT[n,k]) is folded into the
    PSUM->SBUF copy of xT (tensor_scalar multiply), so vlad needs no
    normalized assign tensor: vladT = sum_j AT[j].T @ (xT[j]*rec).
  * assign row-sums come from tiny rhs=[rec,rec] matmuls sharing vlad's
    stationary AT chunks.
  * batches are processed in pairs: the odd batch's vlad/asum matmuls
    target PSUM partitions 64-127 via column tiling (tile_position
    (0,64)), so the centers correction + intra-norm run on full
    128-partition DVE/ACT ops and the final transpose back to [d,k]
    is 4 full 128x128 PE transposes per pair.
  * the second L2 normalization is folded to 1/8 (each of the K=64 unit
    columns contributes 1 to ||desc||^2, so ||desc|| = 8).

bf16 rounding of x/w/assign contributes ~3e-3 relative error, well
inside the 2e-2 gate (measured: see test.py output).
"""

import numpy as np

import concourse.bass as bass
from concourse import bacc
import concourse.mybir as mybir
import concourse.tile as tile
from concourse.bass_utils import run_bass_kernel_spmd
from concourse.masks import make_identity

B, D, K, N = 32, 512, 64, 1024
NCORES = 8
BC = B // NCORES          # batches per core
F32 = mybir.dt.float32
BF16 = mybir.dt.bfloat16
EPS = 1e-12

DC = D // 128             # d chunks (4)
NB = N // 128             # n chunks per batch (8)
NHJ = NB // 2             # n chunks per half (4)


def _netvlad_core(ctx, tc, out, x, w, c):
    """Emit the per-core tile program.

    out: desc [BC, D*K] f32 DRAM     x: [BC, D, N] f32 DRAM
    w:   conv_w [K, D] f32 DRAM      c: centers [D, K] f32 DRAM
    """
    nc = tc.nc
    Exp = mybir.ActivationFunctionType.Exp
    Square = mybir.ActivationFunctionType.Square

    const = ctx.enter_context(tc.tile_pool(name="const", bufs=1))
    xpool = ctx.enter_context(tc.tile_pool(name="xp", bufs=1))
    atp = ctx.enter_context(tc.tile_pool(name="atp", bufs=2))
    sp = ctx.enter_context(tc.tile_pool(name="sp", bufs=2))
    xst = ctx.enter_context(tc.tile_pool(name="xst", bufs=4))
    vp = ctx.enter_context(tc.tile_pool(name="vp", bufs=2))
    op = ctx.enter_context(tc.tile_pool(name="op", bufs=2))
    # PSUM: st(2) + xt(2) + v(2) + o(1) + as(1) = 8 banks
    ps_st = ctx.enter_context(tc.tile_pool(name="ps_st", bufs=2, space="PSUM"))
    ps_xt = ctx.enter_context(tc.tile_pool(name="ps_xt", bufs=2, space="PSUM"))
    ps_v = ctx.enter_context(tc.tile_pool(name="ps_v", bufs=2, space="PSUM"))
    ps_o = ctx.enter_context(tc.tile_pool(name="ps_o", bufs=1, space="PSUM"))
    ps_as = ctx.enter_context(tc.tile_pool(name="ps_as", bufs=1, space="PSUM"))

    # ---- startup-critical ordering on the gpsimd (Q7) queue: the w cast
    # and batch 0's first half emit before the identity builders, which in
    # turn emit before the remaining x halves.  SWDGE descriptor emission
    # is ~3us per half-batch and strictly serial on Q7, so whatever phase 1
    # needs first must be at the queue head.
    wnb = const.tile([K, D], BF16, tag="wnb")
    nc.gpsimd.dma_start(wnb, w)                        # cast f32->bf16 inline
    xb = []
    xsrcs = []
    for b in range(BC):
        xt_ = xpool.tile([128, DC, N], BF16, tag="x", name=f"x{b}", bufs=BC)
        xb.append(xt_)
        xsrcs.append(x[b].rearrange("(cc p) n -> p cc n", p=128))
    h0 = slice(0, 512)
    nc.gpsimd.dma_start(xb[0][:, :, h0], xsrcs[0][:, :, h0])
    identb = const.tile([128, 128], BF16, tag="identb")
    make_identity(nc, identb)
    h1 = slice(512, 1024)
    nc.gpsimd.dma_start(xb[0][:, :, h1], xsrcs[0][:, :, h1])
    ident = const.tile([128, 128], F32, tag="ident")
    make_identity(nc, ident)
    for b in range(1, BC):
        for h in range(2):
            ns = slice(h * 512, (h + 1) * 512)
            nc.gpsimd.dma_start(xb[b][:, :, ns], xsrcs[b][:, :, ns])
    cnat = const.tile([128, DC, K], F32, tag="cnat")
    nc.sync.dma_start(cnat, c.rearrange("(cc p) k -> p cc k", p=128))

    # conv_w^T in bf16: wTb [128(d), 4, 64(k)]
    wT_ps = ps_xt.tile([128, DC, K], BF16, tag="xt", name="wT_ps")
    for cc in range(DC):
        nc.tensor.transpose(
            wT_ps[:, cc, :], wnb[:, cc * 128:(cc + 1) * 128], identb[:K, :K]
        )
    wTb = const.tile([128, DC, K], BF16, tag="wTb")
    nc.vector.tensor_copy(wTb, wT_ps)

    # centers^T replicated on both partition halves: cT2 [128(k2), 512(d)]
    # (regular matmuls, not transpose-mode: walrus requires transpose-MM
    # outputs at PSUM partition 0, and half=1 lands at partition 64;
    # not needed until the first pair epilogue, so emitted late)
    cT2_ps = ps_o.tile([128, DC, 128], F32, tag="o", name="cT2_ps")
    for half in range(2):
        for cc in range(DC):
            nc.tensor.matmul(
                cT2_ps[64 * half:64 * half + 64, cc, :],
                lhsT=cnat[:, cc, :],
                rhs=ident,
            )
    cT2 = const.tile([128, DC, 128], F32, tag="cT2")
    nc.scalar.copy(cT2, cT2_ps)
    cT2f = cT2.rearrange("p cc d -> p (cc d)")

    # assign row-sum accumulators for all 4 batches in one PSUM bank:
    # batch b -> partitions 64*(b%2).., cols 2*(b//2)..
    as_t = ps_as.tile([128, 2 * (BC // 2)], F32, tag="as", name="as_t")

    desc_v = out.rearrange(
        "(bp b2) (cc p k) -> p cc bp b2 k", b2=2, cc=DC, p=128, k=K
    )

    # ---- per batch ----------------------------------------------------
    v2_ps = None
    for b in range(BC):
        bp, b2 = b // 2, b % 2
        base = 64 * b2
        if b2 == 0:
            v2_ps = ps_v.tile([128, 512], F32, tag="v", name=f"v{bp}")

        sT = ps_st.tile([128, NB, K], F32, tag="st", name=f"sT{b}")
        AT = atp.tile([128, NB, K], BF16, tag="AT", name=f"AT{b}")
        red = sp.tile([128, NB], F32, tag="red", name=f"red{b}")
        rec = sp.tile([128, NB], F32, tag="rec", name=f"rec{b}")
        rec2 = sp.tile([128, NB, 2], BF16, tag="rec2", name=f"rec2{b}")

        for h in range(2):
            hs = slice(NHJ * h, NHJ * h + NHJ)
            xt_ps_h = []
            for j in range(NHJ * h, NHJ * h + NHJ):
                xt_ps = ps_xt.tile(
                    [128, DC, 128], BF16, tag="xt", name=f"xt{b}_{j}"
                )
                for cc in range(DC):
                    xchunk = xb[b][:, cc, j * 128:(j + 1) * 128]
                    # scoresT [n,k] accumulated over d chunks
                    nc.tensor.matmul(
                        sT[:, j, :],
                        lhsT=xchunk,
                        rhs=wTb[:, cc, :],
                        start=(cc == 0),
                        stop=(cc == DC - 1),
                    )
                    # xT [n,d] via transpose-mode off the same stationary;
                    # bf16 PSUM halves the downstream copy cost
                    nc.tensor.transpose(xt_ps[:, cc, :], xchunk, identb)
                xt_ps_h.append(xt_ps)

            # softmax pieces for this half (no max-subtraction: scores
            # ~N(0,1) since conv_w is scaled 1/sqrt(D); exp cannot overflow)
            nc.scalar.activation(AT[:, hs, :], sT[:, hs, :], func=Exp)
            nc.vector.tensor_reduce(
                red[:, hs], AT[:, hs, :], axis=mybir.AxisListType.X,
                op=mybir.AluOpType.add,
            )
            nc.vector.reciprocal(rec[:, hs], red[:, hs])
            rh = rec[:, hs]
            rec_bb = bass.AP(
                tensor=rh.tensor, offset=rh.offset,
                ap=[rh.ap[0], rh.ap[1], [0, 2]],
            )
            nc.vector.tensor_copy(rec2[:, hs, :], rec_bb)

            for jj, j in enumerate(range(NHJ * h, NHJ * h + NHJ)):
                # xsT[n,d] = xT[n,d] * rec[n]  (bf16, softmax folded in);
                # alternate DVE / ACT so neither engine stalls the PE's
                # PSUM bank recycling
                xsT = xst.tile(
                    [128, DC, 128], BF16, tag="xs", name=f"xs{b}_{j}", bufs=4
                )
                xs_flat = xsT.rearrange("p cc d -> p (cc d)")
                xt_flat = xt_ps_h[jj].rearrange("p cc d -> p (cc d)")
                if j % 2 == 0:
                    nc.vector.tensor_scalar(
                        xs_flat, xt_flat, rec[:, j:j + 1], None,
                        op0=mybir.AluOpType.mult,
                    )
                else:
                    nc.scalar.activation(
                        xs_flat, xt_flat,
                        func=mybir.ActivationFunctionType.Copy,
                        scale=rec[:, j:j + 1],
                    )
                # vladT [k,d] accumulated over n chunks; odd batch goes to
                # PSUM partitions 64-127 via column tiling
                nc.tensor.matmul(
                    v2_ps[base:base + 64, :],
                    lhsT=AT[:, j, :],
                    rhs=xsT.rearrange("p cc d -> p (cc d)"),
                    start=(j == 0),
                    stop=(j == NB - 1),
                )
                # assign row sums: sum_n AT[n,k]*rec[n]
                nc.tensor.matmul(
                    as_t[base:base + 64, 2 * bp:2 * bp + 2],
                    lhsT=AT[:, j, :],
                    rhs=rec2[:, j, :],
                    start=(j == 0),
                    stop=(j == NB - 1),
                )

        if b2 == 1:
            # ---- pair epilogue: correction, intra-norm, transpose out ----
            # asum negated during the copy so the centers correction fuses
            # into one op: V = cT2*(-asum) + vladT
            asum2 = sp.tile([128, 1], F32, tag="asum", name=f"asum{bp}")
            nc.scalar.mul(asum2, as_t[:, 2 * bp:2 * bp + 1], -1.0)
            V2 = vp.tile([128, 512], F32, tag="V", name=f"V{bp}")
            nc.vector.scalar_tensor_tensor(
                V2, cT2f, asum2, v2_ps,
                op0=mybir.AluOpType.mult, op1=mybir.AluOpType.add,
            )

            sq2 = vp.tile([128, 512], F32, tag="sq", name=f"sq{bp}")
            ss2 = sp.tile([128, 1], F32, tag="ss", name=f"ss{bp}")
            nc.scalar.activation(sq2, V2, func=Square, accum_out=ss2)
            # 1/max(||v||,eps) = exp(-0.5*ln(max(ss,eps^2))): Ln+Exp live in
            # the same ACT table set, unlike Sqrt, so this avoids two 1.3us
            # table reloads per pair
            ssc = sp.tile([128, 1], F32, tag="ssc", name=f"ssc{bp}")
            nc.vector.tensor_scalar_max(ssc, ss2, EPS * EPS)
            lnv = sp.tile([128, 1], F32, tag="lnv", name=f"lnv{bp}")
            nc.scalar.activation(lnv, ssc, func=mybir.ActivationFunctionType.Ln)
            rinv = sp.tile([128, 1], F32, tag="rinv", name=f"rinv{bp}")
            nc.scalar.activation(
                rinv, lnv, func=mybir.ActivationFunctionType.Exp, scale=-0.5
            )
            Vn = vp.tile([128, 512], BF16, tag="Vn", name=f"Vn{bp}")
            nc.vector.tensor_scalar(
                Vn, V2, rinv, 1.0 / 8.0,
                op0=mybir.AluOpType.mult, op1=mybir.AluOpType.mult,
            )

            # transpose [k2, d] -> [d, k2] and store both batches at once
            o_ps = ps_o.tile([128, DC, 128], BF16, tag="o", name=f"o{bp}")
            for cc in range(DC):
                nc.tensor.transpose(
                    o_ps[:, cc, :], Vn[:, cc * 128:(cc + 1) * 128], identb
                )
            o_sb = op.tile([128, DC, 128], F32, tag="osb", name=f"osb{bp}")
            nc.scalar.copy(o_sb, o_ps)
            for b2o in range(2):
                nc.sync.dma_start(
                    desc_v[:, :, bp, b2o, :],
                    o_sb[:, :, b2o * K:(b2o + 1) * K],
                )


_NC_CACHE = None


def _build_nc():
    global _NC_CACHE
    if _NC_CACHE is not None:
        return _NC_CACHE
    from contextlib import ExitStack

    nc = bacc.Bacc("TRN2", target_bir_lowering=False, debug=False,
                   num_devices=NCORES)
    x = nc.dram_tensor("x", [BC, D, N], F32, kind="ExternalInput").ap()
    w = nc.dram_tensor("conv_w", [K, D], F32, kind="ExternalInput").ap()
    c = nc.dram_tensor("centers", [D, K], F32, kind="ExternalInput").ap()
    out = nc.dram_tensor("desc", [BC, D * K], F32, kind="ExternalOutput").ap()
    with tile.TileContext(nc) as tc, ExitStack() as ctx:
        _netvlad_core(ctx, tc, out, x, w, c)
    nc.compile()
    _NC_CACHE = nc
    return nc


def kernel(x, conv_w, centers):
    x = np.ascontiguousarray(x, dtype=np.float32)
    conv_w = np.ascontiguousarray(conv_w, dtype=np.float32)
    centers = np.ascontiguousarray(centers, dtype=np.float32)
    nc = _build_nc()
    in_maps = [
        {
            "x": np.ascontiguousarray(x[i * BC:(i + 1) * BC]),
            "conv_w": conv_w,
            "centers": centers,
        }
        for i in range(NCORES)
    ]
    res = run_bass_kernel_spmd(nc, in_maps, core_ids=list(range(NCORES)))
    return np.concatenate([r["desc"] for r in res.results], axis=0)


# revision 17
# speedup vs baseline: 1.0742x; 1.0742x over previous
"""NetVLAD pooling kernel for Trainium2 (Bass/Tile), 8-core data-parallel.

Reference computation (per batch b):
    scores = conv_w @ x[b]                  # [K, N]
    assign = softmax(scores, axis=K)
    vlad   = x[b] @ assign.T - centers * assign.sum(n)   # [D, K]
    vlad  /= max(||vlad||_2 over D, eps)    # intra-norm per cluster column
    desc   = vlad.reshape(D*K) / max(||.||_2, eps)

Shapes: x [32, 512, 1024] f32, conv_w [64, 512], centers [512, 64],
output desc [32, 32768] f32.  Sharding: data-parallel over batch,
4 batches per core; params replicated.

v2 design (bf16 PE path; v1 was f32r with PE transposes of x and E):

  * x is cast f32->bf16 *during* the DMA (SWDGE on gpsimd), in 8
    half-batch chunks so compute pipelines behind the load.
  * scores are computed TRANSPOSED: sT[n,k] = sum_d x[d,n] wT[d,k] with
    the x chunk [d=128, n=128] as the PE stationary operand.  The same
    stationary chunk then streams the identity to produce xT[n,d] - the
    transpose of x falls out of the weight loads the scores matmul
    already pays for, and the per-batch E^T transposes of v1 vanish
    because softmax-over-k is now a free-dim reduce in [n,k] layout.
  * softmax reciprocal rec[n] = 1/sum_k exp(sT[n,k]) is folded into the
    PSUM->SBUF copy of xT (tensor_scalar multiply), so vlad needs no
    normalized assign tensor: vladT = sum_j AT[j].T @ (xT[j]*rec).
  * assign row-sums come from tiny rhs=[rec,rec] matmuls sharing vlad's
    stationary AT chunks.
  * batches are processed in pairs: the odd batch's vlad/asum matmuls
    target PSUM partitions 64-127 via column tiling (tile_position
    (0,64)), so the centers correction + intra-norm run on full
    128-partition DVE/ACT ops and the final transpose back to [d,k]
    is 4 full 128x128 PE transposes per pair.
  * the second L2 normalization is folded to 1/8 (each of the K=64 unit
    columns contributes 1 to ||desc||^2, so ||desc|| = 8).

bf16 rounding of x/w/assign contributes ~3e-3 relative error, well
inside the 2e-2 gate (measured: see test.py output).
"""

import numpy as np

import concourse.bass as bass
from concourse import bacc
import concourse.mybir as mybir
import concourse.tile as tile
from concourse.bass_utils import run_bass_kernel_spmd
from concourse.masks import make_identity

B, D, K, N = 32, 512, 64, 1024
NCORES = 8
BC = B // NCORES          # batches per core
F32 = mybir.dt.float32
BF16 = mybir.dt.bfloat16
EPS = 1e-12

DC = D // 128             # d chunks (4)
NB = N // 128             # n chunks per batch (8)
NHJ = NB // 2             # n chunks per half (4)


def _netvlad_core(ctx, tc, out, x, w, c):
    """Emit the per-core tile program.

    out: desc [BC, D*K] f32 DRAM     x: [BC, D, N] f32 DRAM
    w:   conv_w [K, D] f32 DRAM      c: centers [D, K] f32 DRAM
    """
    nc = tc.nc
    Exp = mybir.ActivationFunctionType.Exp
    Square = mybir.ActivationFunctionType.Square

    const = ctx.enter_context(tc.tile_pool(name="const", bufs=1))
    xpool = ctx.enter_context(tc.tile_pool(name="xp", bufs=1))
    atp = ctx.enter_context(tc.tile_pool(name="atp", bufs=2))
    sp = ctx.enter_context(tc.tile_pool(name="sp", bufs=2))
    xst = ctx.enter_context(tc.tile_pool(name="xst", bufs=4))
    vp = ctx.enter_context(tc.tile_pool(name="vp", bufs=2))
    op = ctx.enter_context(tc.tile_pool(name="op", bufs=2))
    # PSUM: st(2) + xt(2) + v(2) + o(1) + as(1) = 8 banks
    ps_st = ctx.enter_context(tc.tile_pool(name="ps_st", bufs=2, space="PSUM"))
    ps_xt = ctx.enter_context(tc.tile_pool(name="ps_xt", bufs=2, space="PSUM"))
    ps_v = ctx.enter_context(tc.tile_pool(name="ps_v", bufs=2, space="PSUM"))
    ps_o = ctx.enter_context(tc.tile_pool(name="ps_o", bufs=1, space="PSUM"))
    ps_as = ctx.enter_context(tc.tile_pool(name="ps_as", bufs=1, space="PSUM"))

    # ---- startup-critical ordering on the gpsimd (Q7) queue: the w cast
    # and batch 0's first half emit before the identity builders, which in
    # turn emit before the remaining x halves.  SWDGE descriptor emission
    # is ~3us per half-batch and strictly serial on Q7, so whatever phase 1
    # needs first must be at the queue head.
    wnb = const.tile([K, D], BF16, tag="wnb")
    nc.gpsimd.dma_start(wnb, w)                        # cast f32->bf16 inline
    xb = []
    xsrcs = []
    for b in range(BC):
        xt_ = xpool.tile([128, DC, N], BF16, tag="x", name=f"x{b}", bufs=BC)
        xb.append(xt_)
        xsrcs.append(x[b].rearrange("(cc p) n -> p cc n", p=128))
    h0 = slice(0, 512)
    nc.gpsimd.dma_start(xb[0][:, :, h0], xsrcs[0][:, :, h0])
    identb = const.tile([128, 128], BF16, tag="identb")
    make_identity(nc, identb)
    h1 = slice(512, 1024)
    nc.gpsimd.dma_start(xb[0][:, :, h1], xsrcs[0][:, :, h1])
    ident = const.tile([128, 128], F32, tag="ident")
    make_identity(nc, ident)
    for b in range(1, BC):
        for h in range(2):
            ns = slice(h * 512, (h + 1) * 512)
            nc.gpsimd.dma_start(xb[b][:, :, ns], xsrcs[b][:, :, ns])
    cnat = const.tile([128, DC, K], F32, tag="cnat")
    nc.sync.dma_start(cnat, c.rearrange("(cc p) k -> p cc k", p=128))

    # conv_w^T in bf16: wTb [128(d), 4, 64(k)]
    wT_ps = ps_xt.tile([128, DC, K], BF16, tag="xt", name="wT_ps")
    for cc in range(DC):
        nc.tensor.transpose(
            wT_ps[:, cc, :], wnb[:, cc * 128:(cc + 1) * 128], identb[:K, :K]
        )
    wTb = const.tile([128, DC, K], BF16, tag="wTb")
    nc.vector.tensor_copy(wTb, wT_ps)
    onesb = const.tile([128, 2], BF16, tag="onesb")
    nc.vector.memset(onesb, 1.0)

    # centers^T replicated on both partition halves: cT2 [128(k2), 512(d)]
    # (regular matmuls, not transpose-mode: walrus requires transpose-MM
    # outputs at PSUM partition 0, and half=1 lands at partition 64;
    # not needed until the first pair epilogue, so emitted late)
    cT2_ps = ps_o.tile([128, DC, 128], F32, tag="o", name="cT2_ps")
    for half in range(2):
        for cc in range(DC):
            nc.tensor.matmul(
                cT2_ps[64 * half:64 * half + 64, cc, :],
                lhsT=cnat[:, cc, :],
                rhs=ident,
            )
    cT2 = const.tile([128, DC, 128], F32, tag="cT2")
    nc.scalar.copy(cT2, cT2_ps)
    cT2f = cT2.rearrange("p cc d -> p (cc d)")

    # assign row-sum accumulators for all 4 batches in one PSUM bank:
    # batch b -> partitions 64*(b%2).., cols 2*(b//2)..
    as_t = ps_as.tile([128, 2 * (BC // 2)], F32, tag="as", name="as_t")

    desc_v = out.rearrange(
        "(bp b2) (cc p k) -> p cc bp b2 k", b2=2, cc=DC, p=128, k=K
    )

    # ---- per batch ----------------------------------------------------
    v2_ps = None
    for b in range(BC):
        bp, b2 = b // 2, b % 2
        base = 64 * b2
        if b2 == 0:
            v2_ps = ps_v.tile([128, 512], F32, tag="v", name=f"v{bp}")

        sT = ps_st.tile([128, NB, K], F32, tag="st", name=f"sT{b}")
        AT = atp.tile([128, NB, K], BF16, tag="AT", name=f"AT{b}")
        AN = atp.tile([128, NB, K], BF16, tag="AN", name=f"AN{b}")
        red = sp.tile([128, NB], F32, tag="red", name=f"red{b}")
        rec = sp.tile([128, NB], F32, tag="rec", name=f"rec{b}")

        # per-j pipeline: softmax chain + vlad for chunk j run while the PE
        # streams chunk j+1's score/transpose matmuls, so PE never waits on
        # the DVE/ACT chain at half-batch boundaries
        for j in range(NB):
            xt_ps = ps_xt.tile(
                [128, DC, 128], BF16, tag="xt", name=f"xt{b}_{j}"
            )
            for cc in range(DC):
                xchunk = xb[b][:, cc, j * 128:(j + 1) * 128]
                # scoresT [n,k] accumulated over d chunks
                nc.tensor.matmul(
                    sT[:, j, :],
                    lhsT=xchunk,
                    rhs=wTb[:, cc, :],
                    start=(cc == 0),
                    stop=(cc == DC - 1),
                )
                # xT [n,d] via transpose-mode off the same stationary;
                # bf16 PSUM halves the downstream copy cost
                nc.tensor.transpose(xt_ps[:, cc, :], xchunk, identb)

            # softmax for chunk j (no max-subtraction: scores ~N(0,1)
            # since conv_w is scaled 1/sqrt(D); exp cannot overflow)
            nc.scalar.activation(AT[:, j, :], sT[:, j, :], func=Exp)
            nc.vector.tensor_reduce(
                red[:, j:j + 1], AT[:, j, :], axis=mybir.AxisListType.X,
                op=mybir.AluOpType.add,
            )
            nc.vector.reciprocal(rec[:, j:j + 1], red[:, j:j + 1])
            nc.vector.tensor_scalar(
                AN[:, j, :], AT[:, j, :], rec[:, j:j + 1], None,
                op0=mybir.AluOpType.mult,
            )
            # xT to SBUF as a plain bf16 copy (normalization lives in AN)
            xsT = xst.tile(
                [128, DC, 128], BF16, tag="xs", name=f"xs{b}_{j}", bufs=4
            )
            xs_flat = xsT.rearrange("p cc d -> p (cc d)")
            xt_flat = xt_ps.rearrange("p cc d -> p (cc d)")
            if j % 2 == 0:
                nc.vector.tensor_copy(xs_flat, xt_flat)
            else:
                nc.scalar.copy(xs_flat, xt_flat)
            # vladT [k,d] accumulated over n chunks; odd batch goes to
            # PSUM partitions 64-127 via column tiling
            nc.tensor.matmul(
                v2_ps[base:base + 64, :],
                lhsT=AN[:, j, :],
                rhs=xs_flat,
                start=(j == 0),
                stop=(j == NB - 1),
            )
            # assign row sums: sum_n AN[n,k]
            nc.tensor.matmul(
                as_t[base:base + 64, 2 * bp:2 * bp + 2],
                lhsT=AN[:, j, :],
                rhs=onesb,
                start=(j == 0),
                stop=(j == NB - 1),
            )

        if b2 == 1:
            # ---- pair epilogue: correction, intra-norm, transpose out ----
            # asum negated during the copy so the centers correction fuses
            # into one op: V = cT2*(-asum) + vladT
            asum2 = sp.tile([128, 1], F32, tag="asum", name=f"asum{bp}")
            nc.scalar.mul(asum2, as_t[:, 2 * bp:2 * bp + 1], -1.0)
            V2 = vp.tile([128, 512], F32, tag="V", name=f"V{bp}")
            nc.vector.scalar_tensor_tensor(
                V2, cT2f, asum2, v2_ps,
                op0=mybir.AluOpType.mult, op1=mybir.AluOpType.add,
            )

            sq2 = vp.tile([128, 512], F32, tag="sq", name=f"sq{bp}")
            ss2 = sp.tile([128, 1], F32, tag="ss", name=f"ss{bp}")
            nc.scalar.activation(sq2, V2, func=Square, accum_out=ss2)
            nrm = sp.tile([128, 1], F32, tag="nrm", name=f"nrm{bp}")
            nc.scalar.sqrt(nrm, ss2)
            nrmc = sp.tile([128, 1], F32, tag="nrmc", name=f"nrmc{bp}")
            nc.vector.tensor_scalar_max(nrmc, nrm, EPS)
            rinv = sp.tile([128, 1], F32, tag="rinv", name=f"rinv{bp}")
            nc.vector.reciprocal(rinv, nrmc)
            Vn = vp.tile([128, 512], BF16, tag="Vn", name=f"Vn{bp}")
            nc.vector.tensor_scalar(
                Vn, V2, rinv, 1.0 / 8.0,
                op0=mybir.AluOpType.mult, op1=mybir.AluOpType.mult,
            )

            # transpose [k2, d] -> [d, k2] and store both batches at once
            o_ps = ps_o.tile([128, DC, 128], BF16, tag="o", name=f"o{bp}")
            for cc in range(DC):
                nc.tensor.transpose(
                    o_ps[:, cc, :], Vn[:, cc * 128:(cc + 1) * 128], identb
                )
            o_sb = op.tile([128, DC, 128], F32, tag="osb", name=f"osb{bp}")
            nc.scalar.copy(o_sb, o_ps)
            for b2o in range(2):
                nc.sync.dma_start(
                    desc_v[:, :, bp, b2o, :],
                    o_sb[:, :, b2o * K:(b2o + 1) * K],
                )


_NC_CACHE = None


def _build_nc():
    global _NC_CACHE
    if _NC_CACHE is not None:
        return _NC_CACHE
    from contextlib import ExitStack

    nc = bacc.Bacc("TRN2", target_bir_lowering=False, debug=False,
                   num_devices=NCORES)
    x = nc.dram_tensor("x", [BC, D, N], F32, kind="ExternalInput").ap()
    w = nc.dram_tensor("conv_w", [K, D], F32, kind="ExternalInput").ap()
    c = nc.dram_tensor("centers", [D, K], F32, kind="ExternalInput").ap()
    out = nc.dram_tensor("desc", [BC, D * K], F32, kind="ExternalOutput").ap()
    with tile.TileContext(nc) as tc, ExitStack() as ctx:
        _netvlad_core(ctx, tc, out, x, w, c)
    nc.compile()
    _NC_CACHE = nc
    return nc


def kernel(x, conv_w, centers):
    x = np.ascontiguousarray(x, dtype=np.float32)
    conv_w = np.ascontiguousarray(conv_w, dtype=np.float32)
    centers = np.ascontiguousarray(centers, dtype=np.float32)
    nc = _build_nc()
    in_maps = [
        {
            "x": np.ascontiguousarray(x[i * BC:(i + 1) * BC]),
            "conv_w": conv_w,
            "centers": centers,
        }
        for i in range(NCORES)
    ]
    res = run_bass_kernel_spmd(nc, in_maps, core_ids=list(range(NCORES)))
    return np.concatenate([r["desc"] for r in res.results], axis=0)
